# revision 12
# baseline (speedup 1.0000x reference)
"""Multi-head attention (B=4, S=1024, D=1024, H=16) on 8 Trainium2 NeuronCores.

Sharding: core c handles batch b = c//2 and query-half q = c%2 (512 query
rows).  Each core computes K/V projections for its batch's full sequence
(duplicated across the 2 cores sharing a batch), Q projection + attention +
output projection for its 512 query rows.  The full output is a pure
concatenation of the per-core outputs — no collectives needed.

On-chip layout keeps feature dims on SBUF partitions and token dims on the
free axis ("transposed" activations):
  - projections are matmuls with pre-transposed weights as the stationary
    operand, producing X.T layouts directly;
  - attention scores come out as S^T [k, q] (k on partitions) so the AV
    matmul needs no transposes at all; the per-head score matmuls (contraction
    DK=64) land on PE row-groups (0,0)/(64,0) and run concurrently;
  - softmax denominators come from an extra all-ones column appended to V
    (AV matmul with M=65: rows 0..63 = numerator, row 64 = denominator);
  - softmax is exp(score/8)*mask with no max subtraction (scores are O(1),
    and exp(-65500) == 0 exactly in fp32, so this matches the reference).

Schedule: ~10 dummy matmuls warm the PE clock while the first input chunks
land; V-projection runs on chunked vT loads; per head pair
[Q-proj -> K-proj -> scores/exp/mask with AV trailing 3 k-tiles]; output
projection accumulates head-pair chunks in pair-completion order so its
first chains overlap the last pair's attention tail.

V bias is folded into the output bias on the host (softmax rows sum to 1,
so  x @ Wo.T + bo == x0 @ Wo.T + (bo + Wo @ bv));  Q/K/O biases ride DVE
tensor_scalar adds (per-partition scalars) instead of ACT activations so
the Scalar engine only does the softmax exps.
"""

import numpy as np
import ml_dtypes

import concourse.bass as bass
import concourse.tile as tile
from concourse import bacc, mybir
from concourse import bass_utils

B, S, D, H, DK = 4, 1024, 1024, 16, 64
SQ = S // 2            # query rows per core
NT = D // 128          # 8 partition tiles of the feature dim
NTB = S // 128         # 8 token blocks of the full sequence
NCORES = 8
NPAIR = H // 2         # head pairs (2 heads per 128-partition tile)
BF16 = mybir.dt.bfloat16
F32 = mybir.dt.float32
Exp = mybir.ActivationFunctionType.Exp

_COMPILED = None
TRACE = False
TRACE_CORES = [0]
LAST_RESULT = None

N_WARMUP_MM = 14
BCAST_DMA = False  # stride-0 DMA broadcast rejected by AP checks; use gpsimd


def _emit(nc, tc):
    dram = {n: nc.dram_tensor(n, shp, dt, kind="ExternalInput") for n, shp, dt in [
        ("qT", (NT, 128, SQ), BF16),
        ("kT", (NT, 128, S), BF16),
        ("vT", (NTB, 128, NT, 128), BF16),   # token-block-major for chunked DMA
        ("mT", (NT, 128, SQ), BF16),
        ("wq", (NT, 128, D), BF16),
        ("wk", (NT, 128, D), BF16),
        ("wv", (NT, 128, D), BF16),
        ("wo", (NT, 128, D), BF16),
        ("bqc", (128, NT), F32),
        ("bkc", (128, NT), F32),
        ("boc", (128, NT), F32),
    ]}
    outT = nc.dram_tensor("outT", (NT, 128, SQ), F32, kind="ExternalOutput")

    import contextlib
    stack = contextlib.ExitStack()
    with stack:
        wpool = stack.enter_context(tc.tile_pool(name="wpool", bufs=1))
        inpool = stack.enter_context(tc.tile_pool(name="inpool", bufs=1))
        acts = stack.enter_context(tc.tile_pool(name="acts", bufs=1))
        xpool = stack.enter_context(tc.tile_pool(name="xpool", bufs=1))
        small = stack.enter_context(tc.tile_pool(name="small", bufs=1))
        opool = stack.enter_context(tc.tile_pool(name="opool", bufs=1))
        psS = stack.enter_context(tc.tile_pool(name="psS", bufs=2, space="PSUM"))
        psAV = stack.enter_context(tc.tile_pool(name="psAV", bufs=2, space="PSUM"))
        psU = stack.enter_context(tc.tile_pool(name="psU", bufs=2, space="PSUM"))

        # ---- persistent SBUF tiles ----
        qs = [acts.tile([128, SQ], BF16, name=f"qs{t}") for t in range(NT)]
        ks = [acts.tile([128, S], BF16, name=f"ks{t}") for t in range(NT)]
        vsb = [acts.tile([128, H, DK + 1], BF16, name=f"vsb{t}") for t in range(NT)]
        attnT = [acts.tile([128, SQ], BF16, name=f"attnT{t}") for t in range(NT)]
        vT = inpool.tile([128, NTB, NT, 128], BF16, name="vT")
        kT = inpool.tile([128, NT, S], BF16, name="kT")
        qT = inpool.tile([128, NT, SQ], BF16, name="qT")
        mts = acts.tile([128, NT, SQ], BF16, name="mts")
        # 4 weight tensors cycle 3 slots: wo reuses wv's slot after V-proj
        wvs = wpool.tile([128, NT, D], BF16, tag="wrot", bufs=3, name="wvs")
        wqs = wpool.tile([128, NT, D], BF16, tag="wrot", bufs=3, name="wqs")
        wks = wpool.tile([128, NT, D], BF16, tag="wrot", bufs=3, name="wks")
        wos = wpool.tile([128, NT, D], BF16, tag="wrot", bufs=3, name="wos")

        bq_sb = small.tile([128, NT], F32, name="bq_sb")
        bk_sb = small.tile([128, NT], F32, name="bk_sb")
        bo_sb = small.tile([128, NT], F32, name="bo_sb")
        wz = small.tile([128, 640], BF16, name="wz")

        nc.vector.memset(wz[:], 0.0)
        for t in range(NT):
            nc.vector.memset(vsb[t][:, :, DK:DK + 1], 1.0)

        # ---- PE warmup: ~4 us of dummy matmuls to flip HAM to 8/8 while
        # the first input chunks stream in ----
        wps = psU.tile([128, SQ], F32, tag="psU", name="wps")
        for i in range(N_WARMUP_MM):
            nc.tensor.matmul(wps[:], wz[:, 0:128], wz[:, 128:640],
                             start=True, stop=True)

        # ---- loads: V-projection inputs first, then the rest ----
        def rot(name):
            return dram[name].ap().rearrange("t p f -> p t f")

        nc.scalar.dma_start(bq_sb[:], dram["bqc"].ap())
        nc.scalar.dma_start(bk_sb[:], dram["bkc"].ap())
        nc.scalar.dma_start(bo_sb[:], dram["boc"].ap())
        # sync/gpsimd queues: vT token blocks interleaved, then kT/qT/mask
        for tb in range(0, NTB, 2):
            nc.sync.dma_start(vT[:, tb], dram["vT"].ap()[tb])
            nc.gpsimd.dma_start(vT[:, tb + 1], dram["vT"].ap()[tb + 1])
        nc.sync.dma_start(kT[:, :, 0:512], rot("kT")[:, :, 0:512])
        nc.sync.dma_start(kT[:, :, 512:1024], rot("kT")[:, :, 512:1024])
        nc.gpsimd.dma_start(qT[:], rot("qT"))
        nc.gpsimd.dma_start(mts[:], rot("mT"))
        # scalar queue: wv first (V-proj needs both halves), then wq, wk, wo
        for wname, wsb in (("wv", wvs), ("wq", wqs), ("wk", wks), ("wo", wos)):
            nc.scalar.dma_start(wsb[:, :, 0:512], rot(wname)[:, :, 0:512])
            nc.scalar.dma_start(wsb[:, :, 512:1024], rot(wname)[:, :, 512:1024])

        # ------------- V projection:  vsb[rt][:, h, :] = (value @ Wv.T).T ----
        # both output halves accumulate together so each vT stationary tile
        # is loaded once and feeds two matmuls
        for rt in range(NTB):
            ps0 = psU.tile([128, SQ], F32, tag="psU", name=f"psv{rt}_0")
            ps1 = psU.tile([128, SQ], F32, tag="psU", name=f"psv{rt}_1")
            for dt in range(NT):
                nc.tensor.matmul(ps0[:], vT[:, rt, dt, :], wvs[:, dt, 0:512],
                                 start=(dt == 0), stop=(dt == NT - 1))
                nc.tensor.matmul(ps1[:], vT[:, rt, dt, :], wvs[:, dt, 512:1024],
                                 start=(dt == 0), stop=(dt == NT - 1))
            for half, ps in ((0, ps0), (1, ps1)):
                nc.vector.tensor_copy(
                    vsb[rt][:, half * 8:(half + 1) * 8, 0:DK],
                    ps[:].rearrange("p (h d) -> p h d", h=8))

        # ------------- per head pair: Q-proj, K-proj, scores, AV ----------------
        def make_norm(p, stgs, bcs):
            """Pair p's normalizes, emitted mid-next-pair when their deps
            (broadcast reciprocals) are already satisfied — they never
            head-of-line-block a queue.  GpSimd is otherwise idle."""
            def norm():
                nc.gpsimd.tensor_tensor(attnT[p][0:64, :], stgs[0][0:64, :],
                                        bcs[0][:], mybir.AluOpType.mult)
                st = small.tile([64, SQ], BF16, tag="stg", bufs=2,
                                name=f"stg{2*p+1}")
                nc.gpsimd.tensor_tensor(st[:], stgs[1][0:64, :], bcs[1][:],
                                        mybir.AluOpType.mult)
                nc.sync.dma_start(attnT[p][64:128, :], st[:])
            return norm

        pending_norm = None
        for p in range(NPAIR):
            # Q projection for this pair's o-tile (bias on DVE)
            ps = psU.tile([128, SQ], F32, tag="psU", name=f"psq{p}")
            for dt in range(NT):
                nc.tensor.matmul(ps[:], wqs[:, dt, p * 128:(p + 1) * 128],
                                 qT[:, dt, :], start=(dt == 0),
                                 stop=(dt == NT - 1))
            nc.vector.tensor_scalar_add(qs[p][:], ps[:], bq_sb[:, p:p + 1])

            # K projection: both halves together, one weight load per dt
            psk0 = psU.tile([128, SQ], F32, tag="psU", name=f"psk{p}_0")
            psk1 = psU.tile([128, SQ], F32, tag="psU", name=f"psk{p}_1")
            for dt in range(NT):
                w = wks[:, dt, p * 128:(p + 1) * 128]
                nc.tensor.matmul(psk0[:], w, kT[:, dt, 0:512],
                                 start=(dt == 0), stop=(dt == NT - 1))
                nc.tensor.matmul(psk1[:], w, kT[:, dt, 512:1024],
                                 start=(dt == 0), stop=(dt == NT - 1))
            nc.vector.tensor_scalar_add(ks[p][:, 0:512], psk0[:],
                                        bk_sb[:, p:p + 1])
            nc.vector.tensor_scalar_add(ks[p][:, 512:1024], psk1[:],
                                        bk_sb[:, p:p + 1])

            # scores + exp + mask per k-tile; AV matmuls trail by 3 k-tiles
            exps = []
            avps = [psAV.tile([65, SQ], F32, tag="psAV", name=f"psav{2*p+i}")
                    for i in range(2)]

            def av_mms(kt):
                for i in range(2):
                    nc.tensor.matmul(avps[i][:], vsb[kt][:, 2 * p + i, :],
                                     exps[kt][:, i, :],
                                     start=(kt == 0), stop=(kt == NT - 1))

            for kt in range(NT):
                # AV matmuls first so the two score matmuls stay adjacent in
                # the PE stream and merge onto row-groups (0,0)/(64,0).
                if kt >= 3:
                    av_mms(kt - 3)
                ps = psS.tile([128, S], F32, tag="psS", name=f"pss{p}_{kt}")
                for i, h in enumerate((2 * p, 2 * p + 1)):
                    pbase = (h % 2) * 64
                    lhsT = ks[p][pbase:pbase + 64, kt * 128:(kt + 1) * 128]
                    rhs = qs[p][pbase:pbase + 64, :]
                    nc.tensor.matmul(ps[:, i * SQ:(i + 1) * SQ], lhsT, rhs,
                                     start=True, stop=True)
                ex = xpool.tile([128, 2, SQ], BF16, tag="expS", bufs=12,
                                name=f"ex{p}_{kt}")
                nc.scalar.activation(
                    ex[:], ps[:].rearrange("p (i q) -> p i q", i=2), Exp)
                nc.vector.tensor_tensor(
                    ex[:], ex[:],
                    mts[:, kt, :].unsqueeze(1).to_broadcast((128, 2, SQ)),
                    mybir.AluOpType.mult)
                exps.append(ex)
                if kt == 2 and pending_norm is not None:
                    pending_norm()
                    pending_norm = None
            for kt in range(NT - 3, NT):
                av_mms(kt)

            # Stage each head's AV PSUM to SBUF immediately on ACT (frees the
            # psAV slot for the next pair), 1/den + broadcast right away;
            # only the normalizes are deferred into the next pair.
            stgs, bcs = [], []
            for i in range(2):
                stg = small.tile([65, SQ], F32, tag="avstg", bufs=4,
                                 name=f"avstg{2*p+i}")
                nc.scalar.copy(stg[:], avps[i][:])
                stgs.append(stg)
            for i in range(2):
                h = 2 * p + i
                den0 = small.tile([1, SQ], F32, tag="den0", bufs=2,
                                  name=f"den{h}")
                nc.gpsimd.dma_start(den0[:], stgs[i][64:65, :])
                recip0 = small.tile([1, SQ], F32, tag="recip0", bufs=2,
                                    name=f"recip0_{h}")
                nc.vector.reciprocal_approx_fast(recip0[:], den0[:])
                bc = small.tile([64, SQ], F32, tag="bcast", bufs=4,
                                name=f"bc{h}")
                nc.gpsimd.partition_broadcast(bc[:], recip0[:])
                bcs.append(bc)
            pending_norm = make_norm(p, stgs, bcs)
        pending_norm()
        pending_norm = None

        # ---------------- output projection ----------------
        # dt ascends in pair-completion order, so chains whose psU slot frees
        # during the last pair can accumulate dt=0..6 early.
        for ot in range(NT):
            ps = psU.tile([128, SQ], F32, tag="psU", name=f"pso{ot}")
            for dt in range(NT):
                nc.tensor.matmul(ps[:], wos[:, dt, ot * 128:(ot + 1) * 128],
                                 attnT[dt][:], start=(dt == 0),
                                 stop=(dt == NT - 1))
            osb = opool.tile([128, SQ], F32, tag="osb", bufs=2, name=f"osb{ot}")
            nc.vector.tensor_scalar_add(osb[:], ps[:], bo_sb[:, ot:ot + 1])
            nc.sync.dma_start(outT.ap()[ot], osb[:])


def _build():
    nc = bacc.Bacc("TRN2", target_bir_lowering=False, debug=False,
                   num_devices=NCORES)
    with tile.TileContext(nc) as tc:
        _emit(nc, tc)
    nc.compile()
    return nc


def _get_compiled():
    global _COMPILED
    if _COMPILED is None:
        _COMPILED = _build()
    return _COMPILED


def _tile3(x, dtype=ml_dtypes.bfloat16):
    # [D, N] -> [NT, 128, N] contiguous
    return np.ascontiguousarray(x.reshape(NT, 128, -1)).astype(dtype)


def _tile_tb(xT):
    # [D, S] -> (TB, 128, NT, 128) token-block-major
    return np.ascontiguousarray(
        xT.reshape(NT, 128, NTB, 128).transpose(2, 1, 0, 3)
    ).astype(ml_dtypes.bfloat16)


def kernel(**inputs):
    global LAST_RESULT
    query = np.asarray(inputs["query"], np.float32)
    key = np.asarray(inputs.get("key_in", inputs.get("key"))).astype(np.float32)
    value = np.asarray(inputs["value"], np.float32)
    mask = np.asarray(inputs["mask"])
    Wq = np.asarray(inputs["Wq"], np.float32)
    bq = np.asarray(inputs["bq"], np.float32)
    Wk = np.asarray(inputs["Wk"], np.float32)
    bk = np.asarray(inputs["bk"], np.float32)
    Wv = np.asarray(inputs["Wv"], np.float32)
    bv = np.asarray(inputs["bv"], np.float32)
    Wo = np.asarray(inputs["Wo"], np.float32)
    bo = np.asarray(inputs["bo"], np.float32)

    nc = _get_compiled()

    scale = np.float32(1.0 / np.sqrt(np.float32(DK)))
    bo_eff = bo + Wo @ bv   # softmax rows sum to 1, so bv folds into bo
    shared = {
        "wq": _tile3(Wq.T * scale),       # (Wq/8)^T, d on partitions
        "wk": _tile3(Wk.T),
        "wv": _tile3(Wv.T),
        "wo": _tile3(Wo.T),
        "bqc": np.ascontiguousarray((bq * scale).reshape(NT, 128).T),
        "bkc": np.ascontiguousarray(bk.reshape(NT, 128).T),
        "boc": np.ascontiguousarray(bo_eff.reshape(NT, 128).T),
    }

    in_maps = []
    for c in range(NCORES):
        b, half = divmod(c, 2)
        qsl = slice(half * SQ, (half + 1) * SQ)
        m = dict(shared)
        m["qT"] = _tile3(query[b, qsl].T)
        m["kT"] = _tile3(key[b].T)
        m["vT"] = _tile_tb(value[b].T)
        m["mT"] = _tile3(mask[b, 0, qsl].T.astype(np.float32))
        in_maps.append(m)

    kwargs = {}
    if TRACE:
        kwargs = dict(trace=True, trace_cores=list(TRACE_CORES))
    res = bass_utils.run_bass_kernel_spmd(nc, in_maps,
                                          core_ids=list(range(NCORES)),
                                          **kwargs)
    LAST_RESULT = res

    out = np.empty((B, S, D), np.float32)
    for c in range(NCORES):
        b, half = divmod(c, 2)
        qsl = slice(half * SQ, (half + 1) * SQ)
        oT = res.results[c]["outT"].reshape(D, SQ)
        out[b, qsl] = oT.T
    return out


# revision 17
# speedup vs baseline: 2.1941x; 2.1941x over previous
"""Multi-head attention (B=4, S=1024, D=1024, H=16) on 8 Trainium2 NeuronCores.

Sharding: core c handles batch b = c//2 and query-half q = c%2 (512 query
rows).  Each core computes K/V projections for its batch's full sequence
(duplicated across the 2 cores sharing a batch), Q projection + attention +
output projection for its 512 query rows.  The full output is a pure
concatenation of the per-core outputs — no collectives needed.

On-chip layout keeps feature dims on SBUF partitions and token dims on the
free axis ("transposed" activations):
  - projections are matmuls with pre-transposed weights as the stationary
    operand, producing X.T layouts directly;
  - attention scores come out as S^T [k, q] (k on partitions) so the AV
    matmul needs no transposes at all; the per-head score matmuls (contraction
    DK=64) land on PE row-groups (0,0)/(64,0) and run concurrently;
  - softmax denominators come from an extra all-ones column appended to V
    (AV matmul with M=65: rows 0..63 = numerator, row 64 = denominator);
  - softmax is exp(score/8)*mask with no max subtraction (scores are O(1),
    and exp(-65500) == 0 exactly in fp32, so this matches the reference).

Schedule: ~10 dummy matmuls warm the PE clock while the first input chunks
land; V-projection runs on chunked vT loads; per head pair
[Q-proj -> K-proj -> scores/exp/mask with AV trailing 3 k-tiles]; output
projection accumulates head-pair chunks in pair-completion order so its
first chains overlap the last pair's attention tail.

V bias is folded into the output bias on the host (softmax rows sum to 1,
so  x @ Wo.T + bo == x0 @ Wo.T + (bo + Wo @ bv));  Q/K/O biases ride DVE
tensor_scalar adds (per-partition scalars) instead of ACT activations so
the Scalar engine only does the softmax exps.
"""

import numpy as np
import ml_dtypes

import concourse.bass as bass
import concourse.tile as tile
from concourse import bacc, mybir
from concourse import bass_utils

B, S, D, H, DK = 4, 1024, 1024, 16, 64
SQ = S // 2            # query rows per core
NT = D // 128          # 8 partition tiles of the feature dim
NTB = S // 128         # 8 token blocks of the full sequence
NCORES = 8
NPAIR = H // 2         # head pairs (2 heads per 128-partition tile)
BF16 = mybir.dt.bfloat16
F32 = mybir.dt.float32
Exp = mybir.ActivationFunctionType.Exp

_COMPILED = None
TRACE = False
TRACE_CORES = [0]
LAST_RESULT = None

N_WARMUP_MM = 10
BCAST_DMA = False  # stride-0 DMA broadcast rejected by AP checks; use gpsimd


def _emit(nc, tc):
    dram = {n: nc.dram_tensor(n, shp, dt, kind="ExternalInput") for n, shp, dt in [
        ("qT", (NT, 128, SQ), BF16),
        ("kT", (NT, 128, S), BF16),
        ("vT", (NTB, 128, NT, 128), BF16),   # token-block-major for chunked DMA
        ("mT", (NT, 128, 2, SQ), BF16),
        ("wq", (NT, 128, D), BF16),
        ("wk", (NT, 128, D), BF16),
        ("wv", (NT, 128, D), BF16),
        ("wo", (NT, 128, D), BF16),
        ("bqc", (128, NT), F32),
        ("bkc", (128, NT), F32),
        ("boc", (128, NT), F32),
    ]}
    outT = nc.dram_tensor("outT", (NT, 128, SQ), F32, kind="ExternalOutput")

    import contextlib
    stack = contextlib.ExitStack()
    with stack:
        wpool = stack.enter_context(tc.tile_pool(name="wpool", bufs=1))
        inpool = stack.enter_context(tc.tile_pool(name="inpool", bufs=1))
        acts = stack.enter_context(tc.tile_pool(name="acts", bufs=1))
        xpool = stack.enter_context(tc.tile_pool(name="xpool", bufs=1))
        small = stack.enter_context(tc.tile_pool(name="small", bufs=1))
        opool = stack.enter_context(tc.tile_pool(name="opool", bufs=1))
        psS = stack.enter_context(tc.tile_pool(name="psS", bufs=2, space="PSUM"))
        psAV = stack.enter_context(tc.tile_pool(name="psAV", bufs=2, space="PSUM"))
        psU = stack.enter_context(tc.tile_pool(name="psU", bufs=2, space="PSUM"))

        # ---- persistent SBUF tiles ----
        qs = [acts.tile([128, SQ], BF16, name=f"qs{t}") for t in range(NT)]
        ks = [acts.tile([128, S], BF16, name=f"ks{t}") for t in range(NT)]
        vsb = [acts.tile([128, H, DK + 1], BF16, name=f"vsb{t}") for t in range(NT)]
        attnT = [acts.tile([128, SQ], BF16, name=f"attnT{t}") for t in range(NT)]
        vT = inpool.tile([128, NTB, NT, 128], BF16, name="vT")
        kT = inpool.tile([128, NT, S], BF16, name="kT")
        qT = inpool.tile([128, NT, SQ], BF16, name="qT")
        mts = acts.tile([128, NT, 2, SQ], BF16, name="mts")
        # 4 weight tensors cycle 3 slots: wo reuses wv's slot after V-proj
        wvs = wpool.tile([128, NT, D], BF16, tag="wrot", bufs=3, name="wvs")
        wqs = wpool.tile([128, NT, D], BF16, tag="wrot", bufs=3, name="wqs")
        wks = wpool.tile([128, NT, D], BF16, tag="wrot", bufs=3, name="wks")
        wos = wpool.tile([128, NT, D], BF16, tag="wrot", bufs=3, name="wos")

        bq_sb = small.tile([128, NT], F32, name="bq_sb")
        bk_sb = small.tile([128, NT], F32, name="bk_sb")
        bo_sb = small.tile([128, NT], F32, name="bo_sb")
        wz = small.tile([128, 640], BF16, name="wz")

        nc.vector.memset(wz[:], 0.0)
        for t in range(NT):
            nc.vector.memset(vsb[t][:, :, DK:DK + 1], 1.0)

        # ---- PE warmup: ~4 us of dummy matmuls to flip HAM to 8/8 while
        # the first input chunks stream in ----
        wps = psU.tile([128, SQ], F32, tag="psU", name="wps")
        for i in range(N_WARMUP_MM):
            nc.tensor.matmul(wps[:], wz[:, 0:128], wz[:, 128:640],
                             start=True, stop=True)

        # ---- loads: V-projection inputs first, then the rest ----
        def rot(name):
            return dram[name].ap().rearrange("t p f -> p t f")

        nc.scalar.dma_start(bq_sb[:], dram["bqc"].ap())
        nc.scalar.dma_start(bk_sb[:], dram["bkc"].ap())
        nc.scalar.dma_start(bo_sb[:], dram["boc"].ap())
        # sync queue: wv halves, then kT halves
        nc.sync.dma_start(wvs[:, :, 0:512], rot("wv")[:, :, 0:512])
        nc.sync.dma_start(wvs[:, :, 512:1024], rot("wv")[:, :, 512:1024])
        nc.sync.dma_start(kT[:, :, 0:512], rot("kT")[:, :, 0:512])
        nc.sync.dma_start(kT[:, :, 512:1024], rot("kT")[:, :, 512:1024])
        # gpsimd queue: vT token blocks, then qT and the mask
        for tb in range(NTB):
            nc.gpsimd.dma_start(vT[:, tb], dram["vT"].ap()[tb])
        nc.gpsimd.dma_start(qT[:], rot("qT"))
        nc.gpsimd.dma_start(mts[:], dram["mT"].ap().rearrange("t p i f -> p t i f"))
        # scalar queue (idle until the pair loop): wq, wk, wo halves
        for wname, wsb in (("wq", wqs), ("wk", wks), ("wo", wos)):
            nc.scalar.dma_start(wsb[:, :, 0:512], rot(wname)[:, :, 0:512])
            nc.scalar.dma_start(wsb[:, :, 512:1024], rot(wname)[:, :, 512:1024])

        # ------------- V projection:  vsb[rt][:, h, :] = (value @ Wv.T).T ----
        # half-major so the first 8 chains need only wv's first half
        for half in range(2):
            sl = slice(half * 512, (half + 1) * 512)
            for rt in range(NTB):
                ps = psU.tile([128, SQ], F32, tag="psU", name=f"psv{rt}_{half}")
                for dt in range(NT):
                    nc.tensor.matmul(ps[:], vT[:, rt, dt, :], wvs[:, dt, sl],
                                     start=(dt == 0), stop=(dt == NT - 1))
                nc.vector.tensor_copy(
                    vsb[rt][:, half * 8:(half + 1) * 8, 0:DK],
                    ps[:].rearrange("p (h d) -> p h d", h=8))

        # ------------- per head pair: Q-proj, K-proj, scores, AV ----------------
        def make_norm(p, stgs, bcs):
            """Pair p's normalizes, emitted mid-next-pair when their deps
            (broadcast reciprocals) are already satisfied — they never
            head-of-line-block the DVE queue.  Must NOT go on gpsimd: mixing
            custom-op types there forces ~8us library reloads."""
            def norm():
                nc.vector.tensor_tensor(attnT[p][0:64, :], stgs[0][0:64, :],
                                        bcs[0][:], mybir.AluOpType.mult)
                st = small.tile([64, SQ], BF16, tag="stg", bufs=2,
                                name=f"stg{2*p+1}")
                nc.vector.tensor_tensor(st[:], stgs[1][0:64, :], bcs[1][:],
                                        mybir.AluOpType.mult)
                nc.sync.dma_start(attnT[p][64:128, :], st[:])
            return norm

        pending_norm = None
        for p in range(NPAIR):
            # Q projection for this pair's o-tile (bias on DVE)
            ps = psU.tile([128, SQ], F32, tag="psU", name=f"psq{p}")
            for dt in range(NT):
                nc.tensor.matmul(ps[:], wqs[:, dt, p * 128:(p + 1) * 128],
                                 qT[:, dt, :], start=(dt == 0),
                                 stop=(dt == NT - 1))
            nc.vector.tensor_scalar_add(qs[p][:], ps[:], bq_sb[:, p:p + 1])

            # K projection for this pair's o-tile (two half groups)
            for half in range(2):
                sl = slice(half * 512, (half + 1) * 512)
                ps = psU.tile([128, SQ], F32, tag="psU", name=f"psk{p}_{half}")
                for dt in range(NT):
                    nc.tensor.matmul(ps[:], wks[:, dt, p * 128:(p + 1) * 128],
                                     kT[:, dt, sl], start=(dt == 0),
                                     stop=(dt == NT - 1))
                nc.vector.tensor_scalar_add(ks[p][:, sl], ps[:],
                                            bk_sb[:, p:p + 1])

            # scores + exp + mask per k-tile; AV matmuls trail by 3 k-tiles
            exps = []
            avps = [psAV.tile([65, SQ], F32, tag="psAV", name=f"psav{2*p+i}")
                    for i in range(2)]

            def av_mms(kt):
                for i in range(2):
                    nc.tensor.matmul(avps[i][:], vsb[kt][:, 2 * p + i, :],
                                     exps[kt][:, i, :],
                                     start=(kt == 0), stop=(kt == NT - 1))

            for kt in range(NT):
                # AV matmuls first so the two score matmuls stay adjacent in
                # the PE stream and merge onto row-groups (0,0)/(64,0).
                if kt >= 3:
                    av_mms(kt - 3)
                ps = psS.tile([128, S], F32, tag="psS", name=f"pss{p}_{kt}")
                for i, h in enumerate((2 * p, 2 * p + 1)):
                    pbase = (h % 2) * 64
                    lhsT = ks[p][pbase:pbase + 64, kt * 128:(kt + 1) * 128]
                    rhs = qs[p][pbase:pbase + 64, :]
                    nc.tensor.matmul(ps[:, i * SQ:(i + 1) * SQ], lhsT, rhs,
                                     start=True, stop=True)
                ex = xpool.tile([128, 2, SQ], BF16, tag="expS", bufs=12,
                                name=f"ex{p}_{kt}")
                nc.scalar.activation(
                    ex[:], ps[:].rearrange("p (i q) -> p i q", i=2), Exp)
                nc.vector.tensor_tensor(ex[:], ex[:], mts[:, kt],
                                        mybir.AluOpType.mult)
                exps.append(ex)
                if kt == 2 and pending_norm is not None:
                    pending_norm()
                    pending_norm = None
            for kt in range(NT - 3, NT):
                av_mms(kt)

            # Stage each head's AV PSUM to SBUF immediately on ACT (frees the
            # psAV slot for the next pair), 1/den + broadcast right away;
            # only the normalizes are deferred into the next pair.
            stgs, bcs = [], []
            for i in range(2):
                stg = small.tile([65, SQ], F32, tag="avstg", bufs=4,
                                 name=f"avstg{2*p+i}")
                nc.scalar.copy(stg[:], avps[i][:])
                stgs.append(stg)
            for i in range(2):
                h = 2 * p + i
                den0 = small.tile([1, SQ], F32, tag="den0", bufs=2,
                                  name=f"den{h}")
                nc.gpsimd.dma_start(den0[:], stgs[i][64:65, :])
                recip0 = small.tile([1, SQ], F32, tag="recip0", bufs=2,
                                    name=f"recip0_{h}")
                nc.vector.reciprocal_approx_fast(recip0[:], den0[:])
                bc = small.tile([64, SQ], F32, tag="bcast", bufs=4,
                                name=f"bc{h}")
                nc.gpsimd.partition_broadcast(bc[:], recip0[:])
                bcs.append(bc)
            pending_norm = make_norm(p, stgs, bcs)
        pending_norm()
        pending_norm = None

        # ---------------- output projection ----------------
        # dt ascends in pair-completion order, so chains whose psU slot frees
        # during the last pair can accumulate dt=0..6 early.
        for ot in range(NT):
            ps = psU.tile([128, SQ], F32, tag="psU", name=f"pso{ot}")
            for dt in range(NT):
                nc.tensor.matmul(ps[:], wos[:, dt, ot * 128:(ot + 1) * 128],
                                 attnT[dt][:], start=(dt == 0),
                                 stop=(dt == NT - 1))
            osb = opool.tile([128, SQ], F32, tag="osb", bufs=2, name=f"osb{ot}")
            nc.vector.tensor_scalar_add(osb[:], ps[:], bo_sb[:, ot:ot + 1])
            nc.sync.dma_start(outT.ap()[ot], osb[:])


def _build():
    nc = bacc.Bacc("TRN2", target_bir_lowering=False, debug=False,
                   num_devices=NCORES)
    with tile.TileContext(nc) as tc:
        _emit(nc, tc)
    nc.compile()
    return nc


def _get_compiled():
    global _COMPILED
    if _COMPILED is None:
        _COMPILED = _build()
    return _COMPILED


def _tile3(x, dtype=ml_dtypes.bfloat16):
    # [D, N] -> [NT, 128, N] contiguous
    return np.ascontiguousarray(x.reshape(NT, 128, -1)).astype(dtype)


def _tile_tb(xT):
    # [D, S] -> (TB, 128, NT, 128) token-block-major
    return np.ascontiguousarray(
        xT.reshape(NT, 128, NTB, 128).transpose(2, 1, 0, 3)
    ).astype(ml_dtypes.bfloat16)


def kernel(**inputs):
    global LAST_RESULT
    query = np.asarray(inputs["query"], np.float32)
    key = np.asarray(inputs.get("key_in", inputs.get("key"))).astype(np.float32)
    value = np.asarray(inputs["value"], np.float32)
    mask = np.asarray(inputs["mask"])
    Wq = np.asarray(inputs["Wq"], np.float32)
    bq = np.asarray(inputs["bq"], np.float32)
    Wk = np.asarray(inputs["Wk"], np.float32)
    bk = np.asarray(inputs["bk"], np.float32)
    Wv = np.asarray(inputs["Wv"], np.float32)
    bv = np.asarray(inputs["bv"], np.float32)
    Wo = np.asarray(inputs["Wo"], np.float32)
    bo = np.asarray(inputs["bo"], np.float32)

    nc = _get_compiled()

    scale = np.float32(1.0 / np.sqrt(np.float32(DK)))
    bo_eff = bo + Wo @ bv   # softmax rows sum to 1, so bv folds into bo
    shared = {
        "wq": _tile3(Wq.T * scale),       # (Wq/8)^T, d on partitions
        "wk": _tile3(Wk.T),
        "wv": _tile3(Wv.T),
        "wo": _tile3(Wo.T),
        "bqc": np.ascontiguousarray((bq * scale).reshape(NT, 128).T),
        "bkc": np.ascontiguousarray(bk.reshape(NT, 128).T),
        "boc": np.ascontiguousarray(bo_eff.reshape(NT, 128).T),
    }

    in_maps = []
    for c in range(NCORES):
        b, half = divmod(c, 2)
        qsl = slice(half * SQ, (half + 1) * SQ)
        m = dict(shared)
        m["qT"] = _tile3(query[b, qsl].T)
        m["kT"] = _tile3(key[b].T)
        m["vT"] = _tile_tb(value[b].T)
        mb = _tile3(mask[b, 0, qsl].T.astype(np.float32))
        m["mT"] = np.ascontiguousarray(
            np.broadcast_to(mb[:, :, None, :], (NT, 128, 2, SQ)))
        in_maps.append(m)

    kwargs = {}
    if TRACE:
        kwargs = dict(trace=True, trace_cores=list(TRACE_CORES))
    res = bass_utils.run_bass_kernel_spmd(nc, in_maps,
                                          core_ids=list(range(NCORES)),
                                          **kwargs)
    LAST_RESULT = res

    out = np.empty((B, S, D), np.float32)
    for c in range(NCORES):
        b, half = divmod(c, 2)
        qsl = slice(half * SQ, (half + 1) * SQ)
        oT = res.results[c]["outT"].reshape(D, SQ)
        out[b, qsl] = oT.T
    return out
